# revision 3
# baseline (speedup 1.0000x reference)
"""Supervised-contrastive point-cloud loss on Trainium2 (8 NeuronCores).

Inputs (full): features [8, 128, 4096] f32, labels_all [8, 4096] int32.
Sharding: data-parallel over the batch dim — core b computes the full
4096x4096 per-cloud loss for cloud b; the host averages the 8 scalars.

Per-core algorithm (N=4096 points, C=128 channels, 16 classes):
  v = normalize(f columns)                       (cos matrix prep)
  G = v^T v  in 128-row blocks (bf16 matmuls)
  kill diagonal: G_ii -= 1e5 in PSUM  =>  exp(10*G_ii) underflows to 0
  dp = exp(10 * G)  on the scalar engine (bf16 out)
  CS[c, j] = sum_i onehot[c, i] * dp[i, j]  via a second matmul whose
      stationary operand is the one-hot label matrix (+ a ones row).
      dp is symmetric, so CS[label_j, j] = positives_j and
      CS[16, j] = positives_j + negatives_j.
  dev_j = ln(total_j) - ln(pos_j);  loss = mean_j dev_j
"""

import sys

for _p in ("/opt/trn_rl_repo",):
    if _p not in sys.path:
        sys.path.append(_p)

import numpy as np
import ml_dtypes

import concourse.bass as bass
import concourse.bacc as bacc
import concourse.tile as tile
from concourse import mybir
from concourse.bass_utils import run_bass_kernel_spmd

F32 = mybir.dt.float32
BF16 = mybir.dt.bfloat16
AF = mybir.ActivationFunctionType
ALU = mybir.AluOpType

B, C, N = 8, 128, 4096
NCLS = 16
TEMP_INV = 10.0  # 1 / 0.1
NBLK = N // 128          # 32 row blocks
CHUNK = 1024             # outer column chunk
NCHUNK = N // CHUNK      # 4
BIGDIAG = 1.0e5          # G_ii - 1e5, then exp(10*(..)) == 0.0
NROW = 33                # 16 one-hot rows + 16 pad + totals row at partition 32


def build_program():
    nc = bacc.Bacc("TRN2", target_bir_lowering=False, debug=False, num_devices=B)

    f_d = nc.dram_tensor("f", [C, N], F32, kind="ExternalInput").ap()
    y17_d = nc.dram_tensor("y17", [C, NBLK * NROW], BF16, kind="ExternalInput").ap()
    ybt_d = nc.dram_tensor("ybt", [NCLS, N], F32, kind="ExternalInput").ap()
    bigeye_d = nc.dram_tensor("bigeye", [128, 128], F32, kind="ExternalInput").ap()
    onescol_d = nc.dram_tensor("onescol", [128, 1], F32, kind="ExternalInput").ap()
    onesrow_d = nc.dram_tensor("onesrow", [1, 128], F32, kind="ExternalInput").ap()
    loss_d = nc.dram_tensor("loss", [1, 1], F32, kind="ExternalOutput").ap()

    with tile.TileContext(nc) as tc:
        with (
            tc.tile_pool(name="const", bufs=1) as constp,
            tc.tile_pool(name="big", bufs=1) as bigp,
            tc.tile_pool(name="dp", bufs=3) as dpp,
            tc.tile_pool(name="small", bufs=2) as smallp,
            tc.tile_pool(name="pg", bufs=2, space="PSUM") as pgp,
            tc.tile_pool(name="pcs", bufs=1, space="PSUM") as pcsp,
            tc.tile_pool(name="pmisc", bufs=2, space="PSUM") as pmiscp,
        ):
            # ---- constants / inputs to SBUF ----
            y17_sb = constp.tile([C, NBLK * NROW], BF16)
            nc.sync.dma_start(y17_sb[:], y17_d[:])
            ybt_sb = constp.tile([NCLS, N], F32)
            nc.sync.dma_start(ybt_sb[:], ybt_d[:])
            bigeye_sb = constp.tile([128, 128], F32)
            nc.sync.dma_start(bigeye_sb[:], bigeye_d[:])
            onescol_sb = constp.tile([128, 1], F32)
            nc.sync.dma_start(onescol_sb[:], onescol_d[:])
            onesrow_sb = constp.tile([1, 128], F32)
            nc.sync.dma_start(onesrow_sb[:], onesrow_d[:])

            f_sb = bigp.tile([C, N], F32)
            nc.sync.dma_start(f_sb[:], f_d[:])

            # ---- column norms: s2[j] = sum_c f[c,j]^2 ----
            fsq = bigp.tile([C, N], F32)
            nc.vector.tensor_tensor(fsq[:], f_sb[:], f_sb[:], op=ALU.mult)

            s2row = bigp.tile([1, N], F32)
            for k in range(N // 512):
                s2_ps = pmiscp.tile([1, 512], F32, tag="pm")
                nc.tensor.matmul(
                    s2_ps[:], onescol_sb[:], fsq[:, k * 512 : (k + 1) * 512],
                    start=True, stop=True,
                )
                nc.vector.tensor_scalar_max(
                    s2row[0:1, k * 512 : (k + 1) * 512], s2_ps[:], 1e-24
                )

            # rn = 1/sqrt(s2) = exp(-0.5 * ln(s2)) — ln+exp share one ACT table set
            lnrow = bigp.tile([1, N], F32)
            nc.scalar.activation(lnrow[:], s2row[:], AF.Ln)
            rnrow = bigp.tile([1, N], F32)
            nc.scalar.activation(rnrow[:], lnrow[:], AF.Exp, scale=-0.5)

            # ---- v = f * rn (broadcast rn over partitions via K=1 matmul) ----
            v_sb = bigp.tile([C, N], BF16)
            for k in range(N // 512):
                bc_ps = pmiscp.tile([128, 512], F32, tag="pm")
                nc.tensor.matmul(
                    bc_ps[:], onesrow_sb[:], rnrow[0:1, k * 512 : (k + 1) * 512],
                    start=True, stop=True,
                )
                nc.vector.tensor_tensor(
                    v_sb[:, k * 512 : (k + 1) * 512],
                    f_sb[:, k * 512 : (k + 1) * 512],
                    bc_ps[:],
                    op=ALU.mult,
                )

            # ---- main loop ----
            devrow = bigp.tile([1, N], F32)
            for c in range(NCHUNK):
                c0 = c * CHUNK
                cs = pcsp.tile([NROW, CHUNK], F32)

                def emit_cs(m, dp):
                    lhs = y17_sb[:, m * NROW : (m + 1) * NROW]
                    for h in range(CHUNK // 512):
                        nc.tensor.matmul(
                            cs[:, h * 512 : (h + 1) * 512],
                            lhs,
                            dp[:, h * 512 : (h + 1) * 512],
                            start=(m == 0),
                            stop=(m == NBLK - 1),
                        )

                pending = None
                for m in range(NBLK):
                    g = pgp.tile([128, CHUNK], F32)
                    lhs = v_sb[:, m * 128 : (m + 1) * 128]
                    for h in range(CHUNK // 512):
                        nc.tensor.matmul(
                            g[:, h * 512 : (h + 1) * 512],
                            lhs,
                            v_sb[:, c0 + h * 512 : c0 + (h + 1) * 512],
                            start=True, stop=True,
                        )
                    off = m * 128 - c0
                    if 0 <= off < CHUNK:
                        nc.vector.tensor_tensor(
                            g[:, off : off + 128], g[:, off : off + 128],
                            bigeye_sb[:], op=ALU.subtract,
                        )
                    dp = dpp.tile([128, CHUNK], BF16)
                    nc.scalar.activation(dp[:], g[:], AF.Exp, scale=TEMP_INV)
                    if pending is not None:
                        emit_cs(*pending)
                    pending = (m, dp)
                emit_cs(*pending)

                # ---- positives / totals for this column chunk ----
                cs_sb = smallp.tile([NROW, CHUNK], F32, tag="cssb")
                nc.vector.tensor_copy(cs_sb[:], cs[:])
                masked = smallp.tile([NCLS, CHUNK], F32, tag="msk")
                nc.vector.tensor_tensor(
                    masked[:], cs_sb[0:NCLS, :], ybt_sb[:, c0 : c0 + CHUNK],
                    op=ALU.mult,
                )
                for h in range(CHUNK // 512):
                    pos_ps = pmiscp.tile([1, 512], F32, tag="pm")
                    nc.tensor.matmul(
                        pos_ps[:], onescol_sb[0:NCLS, :],
                        masked[:, h * 512 : (h + 1) * 512],
                        start=True, stop=True,
                    )
                    lp = smallp.tile([1, 512], F32, tag="lp")
                    nc.scalar.activation(lp[:], pos_ps[:], AF.Ln)
                    lt = smallp.tile([1, 512], F32, tag="lt")
                    nc.scalar.activation(
                        lt[:], cs_sb[32:33, h * 512 : (h + 1) * 512], AF.Ln
                    )
                    nc.vector.tensor_tensor(
                        devrow[0:1, c0 + h * 512 : c0 + (h + 1) * 512],
                        lt[:], lp[:], op=ALU.subtract,
                    )

            # ---- mean over points ----
            red = smallp.tile([1, 1], F32, tag="red")
            nc.vector.reduce_sum(red[:], devrow[:], axis=mybir.AxisListType.X)
            lossv = smallp.tile([1, 1], F32, tag="lossv")
            nc.vector.tensor_scalar_mul(lossv[:], red[:], 1.0 / N)
            nc.sync.dma_start(loss_d[:], lossv[:])

    nc.compile()
    return nc


_NC = None


def _get_program():
    global _NC
    if _NC is None:
        _NC = build_program()
    return _NC


def make_in_maps(features, labels_all):
    feats = np.ascontiguousarray(np.asarray(features, dtype=np.float32))
    labels = np.asarray(labels_all, dtype=np.int32)
    onehot = (labels[:, :, None] == np.arange(NCLS)[None, None, :])  # [B, N, 16]
    y17 = np.zeros((B, N, NROW), dtype=ml_dtypes.bfloat16)
    y17[:, :, :NCLS] = onehot
    y17[:, :, NROW - 1] = 1.0
    # [N, NROW] -> [128, NBLK*NROW] so the per-block lhsT slices are contiguous
    y17p = np.ascontiguousarray(
        y17.reshape(B, NBLK, 128, NROW).transpose(0, 2, 1, 3).reshape(B, 128, NBLK * NROW)
    )
    ybt = np.ascontiguousarray(onehot.transpose(0, 2, 1).astype(np.float32))
    bigeye = np.eye(128, dtype=np.float32) * BIGDIAG
    onescol = np.ones((128, 1), np.float32)
    onesrow = np.ones((1, 128), np.float32)
    return [
        {
            "f": feats[b],
            "y17": y17p[b],
            "ybt": ybt[b],
            "bigeye": bigeye,
            "onescol": onescol,
            "onesrow": onesrow,
        }
        for b in range(B)
    ]


def run(features, labels_all, **spmd_kwargs):
    nc = _get_program()
    in_maps = make_in_maps(features, labels_all)
    res = run_bass_kernel_spmd(nc, in_maps, list(range(B)), **spmd_kwargs)
    losses = np.array(
        [res.results[b]["loss"][0, 0] for b in range(B)], dtype=np.float32
    )
    return np.asarray(losses.mean(), dtype=np.float32), res


def kernel(features, labels_all):
    out, _ = run(features, labels_all)
    return out
